# revision 1
# baseline (speedup 1.0000x reference)
"""3-layer GAT decoder on 8 NeuronCores (edge/dst-sharded, Bass/Tile).

Sharding: nodes split into 8 contiguous ranges (N/8 each); core c owns all
edges whose dst lands in its range. Edges are dst-sorted into 128-dst
windows. Node tables are bf16 with 256B-aligned rows; per window, lo/hi-half
dma_gathers in runs of <=8 chunks (SWDGE ring is 1024 descriptors) pull
h[src] rows (al_src appended) from the table. Attention-logit dst terms
broadcast edge-wise via host-precomputed transposed one-hot matrices (fp8,
shared by all 3 layers) matmul'd against the window's al_dst rows; for the
two layers that follow an AllGather this psal term is precomputed for every
window into persistent SBUF tiles during the collective (reverse-order
emission pins the work into the gap). Softmax numerator+denominator
aggregate via one-hot matmuls (one-hots built on DVE with paired-innermost
operands for the 2x mode) accumulated in PSUM; ex is pair-replicated so the
numerator multiply also runs at 2x. Per-window outputs matmul into the next
layer's augmented weights; tables AllGather unsplit (the collective cost
model strongly rewards big transfers). Window metadata (idx16 | paired
dstloc | win_nodes) is packed into one i16 tensor preloaded into persistent
tiles. The SPMD program is identical on all cores.
"""
from contextlib import ExitStack

import numpy as np
import ml_dtypes

import concourse.bass as bass
import concourse.tile as tile
from concourse import bacc, mybir

f32 = mybir.dt.float32
bf16 = mybir.dt.bfloat16
fp8 = mybir.dt.float8e4
i32 = mybir.dt.int32
i16 = mybir.dt.int16
BF = ml_dtypes.bfloat16
F8 = ml_dtypes.float8_e4m3


class Cfg:
    def __init__(self, n_nodes=50000, n_edges=800000, n_cores=8,
                 heads=4, in_ch=64, c1=64, c2=32, out_ch=64):
        self.N = n_nodes
        self.E = n_edges
        self.P = n_cores
        self.H = heads
        self.NC = self.N // self.P
        self.W = (self.NC + 127) // 128
        self.LAST = self.NC - (self.W - 1) * 128
        # per layer: (F_in, C)
        self.layers = [(in_ch, c1), (heads * c1, c2), (heads * c2, out_ch)]
        self.FO = [heads * c1, heads * c2, heads * out_ch]
        # used row content: FO + al_src(H) + al_dst(H)
        self.R = [fo + 2 * heads for fo in self.FO]
        # table row stride, padded to 128-elem (256B bf16) multiple
        self.RP = [((r + 127) // 128) * 128 for r in self.R]
        self.SPLIT = min(32768, ((self.N // 2 + 127) // 128) * 128)
        self.OUT = out_ch
        self.K = None


def host_prep(cfg, x, edge_index, Ws, a_srcs, a_dsts, biases):
    H = cfg.H
    src = np.asarray(edge_index[0], np.int64)
    dst = np.asarray(edge_index[1], np.int64)
    order = np.argsort(dst, kind="stable")
    src_s = src[order].astype(np.int32)
    dst_s = dst[order].astype(np.int32)

    SP = cfg.SPLIT
    per_core = []
    for c in range(cfg.P):
        lo, hi = c * cfg.NC, (c + 1) * cfg.NC
        a = int(np.searchsorted(dst_s, lo))
        b = int(np.searchsorted(dst_s, hi))
        es, ed = src_s[a:b], dst_s[a:b] - lo
        wb = [int(np.searchsorted(ed, w * 128)) for w in range(cfg.W + 1)]
        per_core.append((es, ed, wb))

    cfg.Clo = [0] * cfg.W
    cfg.Chi = [0] * cfg.W
    for c in range(cfg.P):
        es, ed, wb = per_core[c]
        for w in range(cfg.W):
            sl = es[wb[w]:wb[w + 1]]
            nlo = int((sl < SP).sum())
            nhi = len(sl) - nlo
            cfg.Clo[w] = max(cfg.Clo[w], (nlo + 127) // 128)
            cfg.Chi[w] = max(cfg.Chi[w], (nhi + 127) // 128)
    cfg.Kw = [cfg.Clo[w] + cfg.Chi[w] for w in range(cfg.W)]
    cfg.K = int(max(cfg.Kw))
    K = cfg.K

    in_maps = []
    for c in range(cfg.P):
        es, ed, wb = per_core[c]
        pk = np.zeros((cfg.W, 128, K * 10 + 2), np.int16)
        m1aT = np.zeros((cfg.W, 128, K * 128), F8)
        m1a8 = np.zeros((cfg.W, 128, K * 128), F8)
        for w in range(cfg.W):
            a, b = wb[w], wb[w + 1]
            sl = es[a:b]
            dls = (ed[a:b] - w * 128).astype(np.float32)
            mlo = sl < SP
            nlo = int(mlo.sum())
            kw_ = cfg.Kw[w]
            s = np.zeros((kw_ * 128,), np.int32)
            d = np.full((K * 128,), 999.0, np.float32)
            s[:nlo] = sl[mlo]
            d[:nlo] = dls[mlo]
            hoff = cfg.Clo[w] * 128
            s[hoff:hoff + (len(sl) - nlo)] = sl[~mlo] - SP
            d[hoff:hoff + (len(sl) - nlo)] = dls[~mlo]
            dl_w = d.reshape(K, 128).T.astype(BF)       # [128, K]
            m1aT[w] = (d[None, :] ==
                       np.arange(128, dtype=np.float32)[:, None]).astype(F8)
            dmat = d.reshape(K, 128)
            m1a8[w] = (dmat[:, :, None] ==
                       np.arange(128, dtype=np.float32)[None, None, :]
                       ).transpose(1, 0, 2).reshape(128, K * 128).astype(F8)
            wrapped = s.astype(np.int16).reshape(-1, 16).T  # [16, kw_*8]
            for g in range(8):
                pk[w, 16 * g:16 * (g + 1), :kw_ * 8] = wrapped
            dl2 = np.repeat(dl_w, 2, axis=1)            # [128, 2K] paired
            pk[w, :, K * 8:K * 10] = dl2.view(np.int16)
            ids = c * cfg.NC + w * 128 + np.arange(128)
            ids[ids >= (c + 1) * cfg.NC] = 0
            pk[w, :, K * 10:K * 10 + 2] = (
                ids.astype('<i4')[:, None].view(np.int16))
        in_maps.append({"pk": pk, "m1aT": m1aT, "m1a8": m1a8})

    aug = {}
    for li, (W_, asr, adr) in enumerate(zip(Ws, a_srcs, a_dsts)):
        F_in = W_.shape[0]
        C = asr.shape[1]
        Wr = np.asarray(W_, np.float32).reshape(F_in, H, C)
        was = np.einsum("fhc,hc->fh", Wr, np.asarray(asr, np.float32))
        wad = np.einsum("fhc,hc->fh", Wr, np.asarray(adr, np.float32))
        wa = np.concatenate([np.asarray(W_, np.float32), was, wad], 1)
        pad = cfg.RP[li] - wa.shape[1]
        if pad:
            wa = np.concatenate(
                [wa, np.zeros((F_in, pad), np.float32)], 1)
        aug[f"W{li + 1}a"] = wa.astype(BF)

    shared = dict(aug)
    shared["xT"] = np.ascontiguousarray(
        np.asarray(x, np.float32).T.astype(BF))
    shared["iota"] = np.broadcast_to(
        np.arange(128, dtype=np.float32)[None, :], (128, 128)).astype(BF)
    shared["ident"] = np.eye(128, dtype=np.float32).astype(BF)
    for i, b in enumerate(biases):
        dt_np = np.float32 if i == 2 else BF
        shared[f"b{i + 1}r"] = np.broadcast_to(
            np.asarray(b, np.float32)[None, :], (128, len(b))).astype(dt_np)
    for mp in in_maps:
        mp.update(shared)
    return in_maps


def build(cfg, bias_lens):
    H, K, W = cfg.H, cfg.K, cfg.W
    RP = cfg.RP
    SP = cfg.SPLIT
    FO = cfg.FO
    nc = bacc.Bacc("TRN2", target_bir_lowering=False, debug=False,
                   num_devices=cfg.P, num_swdge_queues=4)

    t_xT = nc.dram_tensor("xT", [cfg.layers[0][0], cfg.N], bf16,
                          kind="ExternalInput").ap()
    t_Wa = [nc.dram_tensor(f"W{i + 1}a", [cfg.layers[i][0], RP[i]],
                           bf16, kind="ExternalInput").ap() for i in range(3)]
    t_b = [nc.dram_tensor(f"b{i + 1}r", [128, bias_lens[i]],
                          f32 if i == 2 else bf16,
                          kind="ExternalInput").ap() for i in range(3)]
    t_pk = nc.dram_tensor("pk", [W, 128, K * 10 + 2], i16,
                          kind="ExternalInput").ap()
    t_m1aT = nc.dram_tensor("m1aT", [W, 128, K * 128], fp8,
                            kind="ExternalInput").ap()
    t_m1a8 = nc.dram_tensor("m1a8", [W, 128, K * 128], fp8,
                            kind="ExternalInput").ap()
    t_iota = nc.dram_tensor("iota", [128, 128], bf16,
                            kind="ExternalInput").ap()
    t_id = nc.dram_tensor("ident", [128, 128], bf16,
                          kind="ExternalInput").ap()
    t_out = nc.dram_tensor("recon", [cfg.NC, cfg.OUT], f32,
                           kind="ExternalOutput").ap()

    T1 = nc.dram_tensor("T1", [cfg.N, RP[0]], bf16).ap()
    T2s = nc.dram_tensor("T2s", [cfg.NC, RP[1]], bf16).ap()
    T2 = nc.dram_tensor("T2", [cfg.N, RP[1]], bf16,
                        addr_space="Shared").ap()
    T3s = nc.dram_tensor("T3s", [cfg.NC, RP[2]], bf16).ap()
    T3 = nc.dram_tensor("T3", [cfg.N, RP[2]], bf16,
                        addr_space="Shared").ap()

    NTILE = (cfg.N + 127) // 128

    with tile.TileContext(nc) as tc, ExitStack() as ctx:
        cons = ctx.enter_context(tc.tile_pool(name="cons", bufs=1))
        lpool = ctx.enter_context(tc.tile_pool(name="load", bufs=3))
        gpool = ctx.enter_context(tc.tile_pool(name="gath", bufs=5))
        spool = ctx.enter_context(tc.tile_pool(name="small", bufs=6))
        mpool = ctx.enter_context(tc.tile_pool(name="m1", bufs=4))
        epool = ctx.enter_context(tc.tile_pool(name="epi", bufs=4))
        ppool = ctx.enter_context(tc.tile_pool(name="psals", bufs=1))
        ps_acc = ctx.enter_context(
            tc.tile_pool(name="ps_acc", bufs=2, space="PSUM"))
        ps_tr = ctx.enter_context(
            tc.tile_pool(name="ps_tr", bufs=1, space="PSUM"))
        ps_al = ctx.enter_context(
            tc.tile_pool(name="ps_al", bufs=1, space="PSUM"))
        ps_tb = ctx.enter_context(
            tc.tile_pool(name="ps_tb", bufs=4, space="PSUM"))

        iota_t = cons.tile([128, 128], bf16)
        nc.sync.dma_start(out=iota_t[:], in_=t_iota[:, :])

        ident_t = cons.tile([128, 128], bf16)
        nc.gpsimd.dma_start(out=ident_t[:], in_=t_id[:, :])
        Wa_t = []
        for i in range(3):
            F_in = cfg.layers[i][0]
            parts = []
            for kk in range(0, F_in, 128):
                kw = min(128, F_in - kk)
                wt = cons.tile([kw, RP[i]], bf16, tag=f"W{i}_{kk}")
                if i == 0:
                    nc.sync.dma_start(out=wt[:], in_=t_Wa[i][kk:kk + kw, :])
                else:
                    nc.gpsimd.dma_start(out=wt[:], in_=t_Wa[i][kk:kk + kw, :])
                parts.append(wt)
            Wa_t.append(parts)
        b_t = []
        for i in range(3):
            bt = cons.tile([128, bias_lens[i]], f32 if i == 2 else bf16,
                           tag=f"b{i}")
            nc.sync.dma_start(out=bt[:], in_=t_b[i][:, :])
            b_t.append(bt)

        # ---- phase T1: build table 1 (replicated across cores) ----
        # groups of 4 row-tiles per DMA to amortize the per-DMA queue cost
        NG = NTILE // 4
        for g in range(NG):
            t0 = g * 4
            if g % 2 == 0:
                xt2 = lpool.tile([cfg.layers[0][0], 1024], bf16, tag="xt")
                lim = min(cfg.N, (t0 + 8) * 128) - t0 * 128
                nc.sync.dma_start(out=xt2[:, :lim],
                                  in_=t_xT[:, t0 * 128:t0 * 128 + lim])
            xt = xt2[:, (g % 2) * 512:(g % 2) * 512 + 512]
            hrow = lpool.tile([128, 4, RP[0]], bf16, tag="hrow")
            R0 = cfg.R[0]
            for k in range(4):
                ps = ps_tb.tile([128, RP[0]], f32, tag="tb")
                nc.tensor.matmul(out=ps[:, :R0],
                                 lhsT=xt[:, k * 128:(k + 1) * 128],
                                 rhs=Wa_t[0][0][:, :R0], start=True, stop=True)
                if k % 2 == 0:
                    nc.vector.tensor_copy(hrow[:, k, :R0], ps[:, :R0])
                else:
                    nc.scalar.activation(
                        hrow[:, k, :R0], ps[:, :R0],
                        func=mybir.ActivationFunctionType.Copy)
            eng = nc.sync
            eng.dma_start(
                out=T1[t0 * 128:(t0 + 4) * 128, :cfg.R[0]].rearrange(
                    "(k p) r -> p k r", k=4),
                in_=hrow[:, :, :cfg.R[0]])
        for t in range(NG * 4, NTILE):
            tr_ = min(128, cfg.N - t * 128)
            xt1 = lpool.tile([cfg.layers[0][0], 128], bf16, tag="xt1")
            nc.sync.dma_start(out=xt1[:, :tr_],
                              in_=t_xT[:, t * 128:t * 128 + tr_])
            ps = ps_tb.tile([128, RP[0]], f32, tag="tb")
            nc.tensor.matmul(out=ps[:tr_, :cfg.R[0]], lhsT=xt1[:, :tr_],
                             rhs=Wa_t[0][0][:, :cfg.R[0]],
                             start=True, stop=True)
            hrow1 = lpool.tile([128, RP[0]], bf16, tag="hrow1")
            nc.vector.tensor_copy(hrow1[:tr_, :cfg.R[0]],
                                  ps[:tr_, :cfg.R[0]])
            nc.sync.dma_start(out=T1[t * 128:t * 128 + tr_, :cfg.R[0]],
                              in_=hrow1[:tr_, :cfg.R[0]])

        pk_t = []
        for w in range(W):
            pt = cons.tile([128, K * 10 + 2], i16, tag=f"pk{w}")
            nc.gpsimd.dma_start(out=pt[:], in_=t_pk[w])
            pk_t.append(pt)

        tables = [T1, T2, T3]
        slices = [T2s, T3s, None]
        fulls = [T2, T3, None]
        for li in range(3):
            Kw, Clo = cfg.Kw, cfg.Clo
            F_in, C = cfg.layers[li]
            FOl = FO[li]
            AW = FOl + H
            Tbl = tables[li]
            cat = li < 2
            # For layers after an AllGather, psal (per-edge al_dst terms)
            # depends only on core-local data — precompute it for every
            # window while the collective runs, into persistent SBUF tiles.
            pre = None
            if li > 0:
                pre = [None] * W
                for w in range(W - 1, -1, -1):
                    rows = 128 if w < W - 1 else cfg.LAST
                    base = w * 128
                    KW_ = Kw[w]
                    alwp = spool.tile([128, H], bf16, tag="alw")
                    if rows < 128:
                        nc.vector.memset(alwp[:], 0.0)
                    nc.sync.dma_start(
                        out=alwp[:rows, :],
                        in_=slices[li - 1][base:base + rows,
                                           FOl + H:FOl + 2 * H])
                    m1aTp = mpool.tile([128, K * 128], fp8, tag="m1aT")
                    nc.sync.dma_start(out=m1aTp[:, :KW_ * 128],
                                      in_=t_m1aT[w, :, :KW_ * 128])
                    psq = ps_al.tile([128, K * H], f32, tag="psal")
                    for j in range(KW_):
                        nc.tensor.matmul(out=psq[:, j * H:(j + 1) * H],
                                         lhsT=m1aTp[:, j * 128:(j + 1) * 128],
                                         rhs=alwp[:],
                                         start=True, stop=True)
                    psb = ppool.tile([128, K * H], bf16, tag=f"psb{w}")
                    if w % 2 == 0:
                        nc.vector.tensor_copy(psb[:, :KW_ * H],
                                              psq[:, :KW_ * H])
                    else:
                        nc.scalar.activation(
                            psb[:, :KW_ * H], psq[:, :KW_ * H],
                            func=mybir.ActivationFunctionType.Copy)
                    pre[w] = psb
            for w in range(W):
                rows = 128 if w < W - 1 else cfg.LAST
                base = w * 128
                KW_ = Kw[w]
                pkw = pk_t[w]
                idxw = pkw[:, :K * 8]
                dl2 = pkw[:, K * 8:K * 10].bitcast(bf16)
                if li == 0:
                    m1aT = mpool.tile([128, K * 128], fp8, tag="m1aT")
                    nc.sync.dma_start(out=m1aT[:, :KW_ * 128],
                                        in_=t_m1aT[w, :, :KW_ * 128])
                    alw = spool.tile([128, H], bf16, tag="alw")
                    wnw = pkw[:, K * 10:K * 10 + 2].bitcast(i32)
                    nc.gpsimd.indirect_dma_start(
                        out=alw[:], out_offset=None, in_=Tbl[:, :],
                        in_offset=bass.IndirectOffsetOnAxis(
                            ap=wnw[:, :], axis=0),
                        element_offset=FOl + H)

                gath = gpool.tile([128, K, RP[li]], bf16, tag="g")
                if li == 0:
                    m1a = mpool.tile([128, K * 128], bf16, tag="m1a")
                else:
                    m1a = mpool.tile([128, K * 128], fp8, tag="m1aT")
                    nc.sync.dma_start(out=m1a[:, :KW_ * 128],
                                      in_=t_m1a8[w, :, :KW_ * 128])
                rhs = gpool.tile([128, K, AW], bf16, tag="rhs")
                if li == 0:
                    psal = ps_al.tile([128, K * H], f32, tag="psal")
                acc = ps_acc.tile([128, AW], f32, tag="acc")

                # one gather per table half, rotating SWDGE queues
                runs = []
                for a0, b0 in ((0, Clo[w]), (Clo[w], KW_)):
                    a = a0
                    while a < b0:
                        b = min(a + 8, b0)
                        runs.append((a, b, a0 > 0))
                        a = b
                for (a, b, is_hi) in runs:
                    tbl_view = Tbl[SP:cfg.N, :] if is_hi else Tbl[0:SP, :]
                    nc.gpsimd.dma_gather(
                        out_ap=gath[:, a:b, :],
                        in_ap=tbl_view,
                        idxs_ap=idxw[:, a * 8:b * 8],
                        num_idxs=(b - a) * 128,
                        num_idxs_reg=(b - a) * 128,
                        elem_size=RP[li],
                        queue_num=0,
                    )
                # m1a one-hot (edge-partition, dst-free): built on DVE for
                # L1 (DMA-bound phase); host-loaded fp8 for L2/L3 where DVE
                # or chain latency binds (one-hots are layer-independent)
                if li == 0:
                    nc.vector.tensor_tensor(
                        out=m1a[:, :KW_ * 128].rearrange(
                            "p (k a b) -> p k a b", a=64, b=2),
                        in0=iota_t.rearrange("p (a b) -> p a b", b=2)[
                            :, None, :, :].to_broadcast([128, KW_, 64, 2]),
                        in1=dl2.rearrange("p (k b) -> p k b", b=2)[
                            :, :KW_, None, :].to_broadcast(
                            [128, KW_, 64, 2]),
                        op=mybir.AluOpType.is_equal)
                # al_dst edge-broadcast: psal[:, j*H:(j+1)*H] = m1aT_j^T @ alw
                if li == 0:
                    for j in range(KW_):
                        nc.tensor.matmul(out=psal[:, j * H:(j + 1) * H],
                                         lhsT=m1aT[:, j * 128:(j + 1) * 128],
                                         rhs=alw[:],
                                         start=True, stop=True)
                    al_d = psal[:, :KW_ * H].rearrange(
                        "p (k h) -> p k h", h=H)
                else:
                    al_d = pre[w][:, :KW_ * H].rearrange(
                        "p (k h) -> p k h", h=H)

                e_all = spool.tile([128, K, H], f32, tag="eall")
                nc.vector.tensor_tensor(
                    out=e_all[:, :KW_, :],
                    in0=gath[:, :KW_, FOl:FOl + H],
                    in1=al_d,
                    op=mybir.AluOpType.add)
                # leaky relu in one fused op: max(0.2*e, e)
                e_lr = spool.tile([128, K, H], f32, tag="elr")
                nc.vector.scalar_tensor_tensor(
                    out=e_lr[:, :KW_, :], in0=e_all[:, :KW_, :], scalar=0.2,
                    in1=e_all[:, :KW_, :], op0=mybir.AluOpType.mult,
                    op1=mybir.AluOpType.max)
                # exp straight into rhs's denominator slot (bf16 cast)
                nc.scalar.activation(
                    rhs[:, :KW_, FOl:FOl + H], e_lr[:, :KW_, :],
                    func=mybir.ActivationFunctionType.Exp)
                # duplicate ex into adjacent pairs so the numerator mult has
                # stride-1 innermost on every operand (2x DVE mode)
                exb2 = spool.tile([128, K, H, 2], bf16, tag="exb2")
                nc.vector.tensor_copy(
                    exb2[:, :KW_, :, :],
                    rhs[:, :KW_, FOl:FOl + H][:, :, :, None].to_broadcast(
                        [128, KW_, H, 2]))
                nc.vector.tensor_tensor(
                    out=rhs[:, :KW_, 0:FOl].rearrange(
                        "p k (h a b) -> p k h a b", h=H, b=2),
                    in0=gath[:, :KW_, 0:FOl].rearrange(
                        "p k (h a b) -> p k h a b", h=H, b=2),
                    in1=exb2[:, :KW_, :, None, :].to_broadcast(
                        [128, KW_, H, C // 2, 2]),
                    op=mybir.AluOpType.mult)
                for j in range(KW_):
                    nc.tensor.matmul(out=acc[:],
                                     lhsT=m1a[:, j * 128:(j + 1) * 128],
                                     rhs=rhs[:, j, :],
                                     start=(j == 0), stop=(j == KW_ - 1))

                s_t = spool.tile([128, H], f32, tag="st")
                if cat:
                    nc.vector.tensor_scalar(
                        out=s_t[:], in0=acc[:, FOl:FOl + H],
                        scalar1=1e-16, scalar2=None,
                        op0=mybir.AluOpType.add)
                else:
                    # fold the head-mean 1/H into the reciprocal
                    nc.vector.tensor_scalar(
                        out=s_t[:], in0=acc[:, FOl:FOl + H],
                        scalar1=1e-16, scalar2=float(H),
                        op0=mybir.AluOpType.add,
                        op1=mybir.AluOpType.mult)
                rcp = spool.tile([128, H], f32, tag="rcp")
                nc.vector.reciprocal(rcp[:], s_t[:])
                if cat:
                    outw = epool.tile([128, FOl], bf16, tag="outw")
                    nc.vector.tensor_tensor(
                        out=outw.rearrange("p (h c) -> p h c", h=H),
                        in0=acc[:, 0:FOl].rearrange("p (h c) -> p h c", h=H),
                        in1=rcp[:, :, None].to_broadcast([128, H, C]),
                        op=mybir.AluOpType.mult)
                    nc.vector.tensor_tensor(out=outw[:], in0=outw[:],
                                            in1=b_t[li][:],
                                            op=mybir.AluOpType.add)
                    nk = FOl // 128
                    pst2 = ps_tr.tile([128, FOl], bf16, tag="pst")
                    for kk in range(nk):
                        nc.tensor.transpose(
                            out=pst2[:, kk * 128:(kk + 1) * 128],
                            in_=outw[:, kk * 128:(kk + 1) * 128],
                            identity=ident_t[:])
                    otr = epool.tile([128, FOl], bf16, tag="otr")
                    nc.scalar.activation(otr[:], pst2[:],
                                         func=mybir.ActivationFunctionType.Copy)
                    Rn = RP[li + 1]
                    Ru = cfg.R[li + 1]
                    psn = ps_tb.tile([128, Rn], f32, tag="tb")
                    for pi in range(nk):
                        nc.tensor.matmul(out=psn[:],
                                         lhsT=otr[:, pi * 128:(pi + 1) * 128],
                                         rhs=Wa_t[li + 1][pi][:],
                                         start=(pi == 0),
                                         stop=(pi == nk - 1))
                    hnext = epool.tile([128, Rn], bf16, tag="hnext")
                    nc.scalar.activation(hnext[:], psn[:],
                                         func=mybir.ActivationFunctionType.Copy)
                    nc.sync.dma_start(
                        out=slices[li][base:base + rows, :Ru],
                        in_=hnext[:rows, :Ru])
                else:
                    msum = epool.tile([128, C, H], f32, tag="msum")
                    for h in range(H):
                        if h % 2 == 0:
                            nc.scalar.activation(
                                msum[:, :, h],
                                acc[:, h * C:(h + 1) * C],
                                func=mybir.ActivationFunctionType.Copy,
                                scale=rcp[:, h:h + 1])
                        else:
                            nc.vector.tensor_scalar(
                                out=msum[:, :, h],
                                in0=acc[:, h * C:(h + 1) * C],
                                scalar1=rcp[:, h:h + 1], scalar2=None,
                                op0=mybir.AluOpType.mult)
                    outm = epool.tile([128, C], f32, tag="outm")
                    nc.vector.tensor_reduce(
                        out=outm[:],
                        in_=msum[:],
                        axis=mybir.AxisListType.X,
                        op=mybir.AluOpType.add)
                    nc.vector.tensor_tensor(
                        out=outm[:], in0=outm[:], in1=b_t[li][:],
                        op=mybir.AluOpType.add)
                    nc.sync.dma_start(out=t_out[base:base + rows, :],
                                      in_=outm[:rows, :])
            if li < 2:
                nc.gpsimd.collective_compute(
                    "AllGather", mybir.AluOpType.bypass,
                    ins=[slices[li][:, :]], outs=[fulls[li][:, :]],
                    replica_groups=[list(range(cfg.P))],
                )
    nc.compile()
    return nc


def run(cfg, inputs, hw=True):
    """inputs: raw reference inputs dict. Returns [N, OUT] output."""
    in_maps = host_prep(
        cfg, inputs["x"], inputs["edge_index"],
        [inputs["W1"], inputs["W2"], inputs["W3"]],
        [inputs["a1_src"], inputs["a2_src"], inputs["a3_src"]],
        [inputs["a1_dst"], inputs["a2_dst"], inputs["a3_dst"]],
        [inputs["b1"], inputs["b2"], inputs["b3"]],
    )
    bias_lens = [len(inputs["b1"]), len(inputs["b2"]), len(inputs["b3"])]
    nc = build(cfg, bias_lens)
    if hw:
        from concourse.bass_utils import run_bass_kernel_spmd
        res = run_bass_kernel_spmd(nc, in_maps, list(range(cfg.P)))
        out = np.concatenate([res.results[c]["recon"] for c in range(cfg.P)], 0)
        return out, res
    else:
        from concourse.bass_interp import MultiCoreSim
        sim = MultiCoreSim(nc, cfg.P)
        for c in range(cfg.P):
            for k, v in in_maps[c].items():
                sim.cores[c].tensor(k)[:] = v
        sim.simulate()
        out = np.concatenate(
            [np.array(sim.cores[c].tensor("recon")) for c in range(cfg.P)], 0)
        return out, sim


# ---------------------------------------------------------------------------
# Harness entry point: kernel(**inputs) -> full [50000, 64] output.
# ---------------------------------------------------------------------------
import os as _os

last_exec_time_ns = None
_build_cache = {}


def kernel(**inputs):
    global last_exec_time_ns
    cfg = Cfg()
    in_maps = host_prep(
        cfg, inputs["x"], inputs["edge_index"],
        [inputs["W1"], inputs["W2"], inputs["W3"]],
        [inputs["a1_src"], inputs["a2_src"], inputs["a3_src"]],
        [inputs["a1_dst"], inputs["a2_dst"], inputs["a3_dst"]],
        [inputs["b1"], inputs["b2"], inputs["b3"]],
    )
    bias_lens = (len(inputs["b1"]), len(inputs["b2"]), len(inputs["b3"]))
    key = (cfg.K, bias_lens)
    if key not in _build_cache:
        _build_cache[key] = build(cfg, list(bias_lens))
    nc = _build_cache[key]

    from concourse.bass_utils import run_bass_kernel_spmd
    trace = _os.environ.get("KERNEL_TRACE", "0") == "1"
    res = run_bass_kernel_spmd(nc, in_maps, list(range(cfg.P)), trace=trace)
    last_exec_time_ns = res.exec_time_ns
    out = np.concatenate([res.results[c]["recon"] for c in range(cfg.P)], 0)
    return out.astype(np.float32)

